# revision 17
# baseline (speedup 1.0000x reference)
"""Trainium2 Bass kernel for nn_HardcodedKVMemoryBlock (8 NeuronCores).

Sharding: core i handles batch b=i//4, sequence chunk c=i%4 (512 tokens).
The (B,L,P,D) cumsum is restructured as causal linear attention:
    retrieved = tril(Q K^T) @ V + Q @ S_prefix
with Q=[cos,sin] phasors (L x 64), V = values at odd positions, and the
cross-chunk carry S_prefix = (K_even^T @ x_odd) @ val_W computed
redundantly per core from a zero-padded prefix (no collectives).
The 1/sqrt(valid*P) normalization cancels inside LayerNorm1 (scale
invariance); ln gains are folded into W1/Wo and means are applied as
rank-1 PE updates, so LN costs no extra full-width element-wise passes
beyond one multiply by the broadcast rstd.
"""

import math
import numpy as np
import ml_dtypes

import concourse.bass as bass
import concourse.tile as tile
from concourse import bacc, mybir
from concourse.bass_utils import run_bass_kernel_spmd

PI = math.pi
B, L, D, P = 2, 2048, 256, 32
T = 512          # own tokens per core
H = 512          # MLP hidden
PRE = 768        # padded prefix pair count (max prefix 1536 tokens / 2)
N_CORES = 8

f32 = mybir.dt.float32
f32r = mybir.dt.float32r
bf16 = mybir.dt.bfloat16
AF = mybir.ActivationFunctionType
OP = mybir.AluOpType


def _r(ap):
    return ap.bitcast(f32r)


def _build():
    nc = bacc.Bacc("TRN2", target_bir_lowering=False, debug=False,
                   num_devices=N_CORES)

    def din(name, shape, dt):
        return nc.dram_tensor(name, shape, dt, kind="ExternalInput").ap()

    xc = din("xc", [T, D], f32)
    xpe = din("xpe", [PRE, D], bf16)
    xpo = din("xpo", [PRE, D], bf16)
    kw = din("kw", [D, P], f32r)
    kwb = din("kwb", [D, P], bf16)
    kbc = din("kbc", [P, 1], f32)
    vw = din("vw", [D, D], f32r)
    w1g = din("w1g", [D, H], f32r)
    c1n = din("c1n", [1, H], f32r)
    cb1 = din("cb1", [H, 1], f32)
    w2 = din("w2", [H, D], f32r)
    b2c = din("b2c", [D, 1], f32)
    wog = din("wog", [D, D], f32r)
    c1on = din("c1on", [1, D], f32r)
    m0 = din("m0", [128, 256], f32)
    onesr = din("onesr", [1, 128], f32r)
    invd = din("invd", [128, 1], f32r)
    eyef = din("eyef", [128, 128], f32)
    eyeb = din("eyeb", [128, 128], bf16)
    outc = nc.dram_tensor("outc", [T, D], f32, kind="ExternalOutput").ap()

    with tile.TileContext(nc) as tc:
        _emit(tc, locals())
    nc.compile()
    return nc


def _emit(tc, io):
    nc = tc.nc
    xc, xpe, xpo = io["xc"], io["xpe"], io["xpo"]
    outc = io["outc"]

    sb = tc.alloc_tile_pool(name="sb", bufs=1)
    pt = tc.alloc_tile_pool(name="pt", bufs=2, space="PSUM")    # transposes
    pa = tc.alloc_tile_pool(name="pa", bufs=2, space="PSUM")    # narrow tiles
    pb = tc.alloc_tile_pool(name="pb", bufs=3, space="PSUM")    # [128,512]

    # ---------------- constant / weight tiles ----------------
    kw_sb = sb.tile([128, 2 * P], f32r)       # ktile kt at cols [P*kt]
    kwb_sb = sb.tile([128, 2 * P], bf16)
    kbc_sb = sb.tile([P, 1], f32)
    vw_sb = sb.tile([128, 512], f32r)         # ktile kt at cols [256*kt]
    w1g_sb = sb.tile([128, 1024], f32r)      # ktile kt at cols [512*kt]
    c1n_sb = sb.tile([1, H], f32r)
    cb1_sb = sb.tile([128, 4], f32)          # mtile m at col m
    w2_sb = sb.tile([128, 1024], f32r)       # ktile kt at cols [256*kt]
    b2c_sb = sb.tile([128, 2], f32)
    wog_sb = sb.tile([128, 512], f32r)       # ktile dh at cols [256*dh]
    c1on_sb = sb.tile([1, D], f32r)
    m0_sb = sb.tile([128, 256], f32)
    eyef_sb = sb.tile([128, 128], f32)
    eyeb_sb = sb.tile([128, 128], bf16)
    ones_sb = sb.tile([1, 128], f32r)
    invd_sb = sb.tile([128, 1], f32r)
    epsb_sb = sb.tile([128, 1], f32)
    halfpi_sb = sb.tile([P, 1], f32)
    zerop_sb = sb.tile([P, 1], f32)
    zero128_sb = sb.tile([128, 1], f32)
    zero1_sb = sb.tile([1, 1], f32)

    dma = nc.sync.dma_start
    dma(kw_sb[:], io["kw"].rearrange("(k p) q -> p k q", p=128))
    dma(kwb_sb[:], io["kwb"].rearrange("(k p) q -> p k q", p=128))
    dma(kbc_sb[:], io["kbc"])
    dma(vw_sb[:], io["vw"].rearrange("(k p) q -> p k q", p=128))
    dma(w1g_sb[:], io["w1g"].rearrange("(k p) q -> p k q", p=128))
    dma(c1n_sb[:], io["c1n"])
    dma(cb1_sb[:], io["cb1"].rearrange("(m p) o -> p m o", p=128))
    dma(w2_sb[:], io["w2"].rearrange("(k p) q -> p k q", p=128))
    dma(b2c_sb[:], io["b2c"].rearrange("(m p) o -> p m o", p=128))
    dma(wog_sb[:], io["wog"].rearrange("(k p) q -> p k q", p=128))
    dma(c1on_sb[:], io["c1on"])
    dma(m0_sb[:], io["m0"])
    dma(eyef_sb[:], io["eyef"])
    dma(eyeb_sb[:], io["eyeb"])
    dma(ones_sb[:], io["onesr"])
    dma(invd_sb[:], io["invd"])
    nc.vector.memset(epsb_sb[:], 1e-5)
    nc.vector.memset(halfpi_sb[:], PI / 2)
    nc.vector.memset(zerop_sb[:], 0.0)
    nc.vector.memset(zero128_sb[:], 0.0)
    nc.vector.memset(zero1_sb[:], 0.0)

    # ---------------- data tiles ----------------
    xc_sb = sb.tile([128, 1024], f32)        # token tile tt at cols [256*tt]
    xT_sb = sb.tile([128, 1024], f32r)        # d-half dh at cols [512*dh]
    xpe_sb = sb.tile([128, 1536], bf16)      # block j at cols [256*j]
    xpo_sb = sb.tile([128, 1536], bf16)
    xpeT_sb = sb.tile([128, 1536], bf16)     # ktile kt at cols [768*kt]
    t_sb = sb.tile([P, T], f32)
    ta_sb = sb.tile([P, T], f32)
    qb_sb = sb.tile([2 * P, T], f32r)         # rows 0:32 cos, 32:64 sin
    tpre_sb = sb.tile([P, PRE], f32)
    tpa_sb = sb.tile([P, PRE], f32)
    kpre_sb = sb.tile([2 * P, PRE], bf16)
    kpreT_sb = sb.tile([128, 6 * 64], bf16)  # block j at cols [64*j]
    g_sb = sb.tile([2 * P, D], f32)
    gT_sb = sb.tile([128, 128], f32r)         # ktile dh at cols [64*dh]
    s_sb = sb.tile([2 * P, D], f32r)
    vodd_sb = sb.tile([128, 512], f32r)       # block blk at cols [256*blk]
    ss0_sb = sb.tile([128, 512], f32r)
    ss1_sb = sb.tile([128, 256], f32r)
    r_sb = sb.tile([128, 1024], f32r)         # retrieved, dh at cols [512*dh]
    sq_sb = sb.tile([128, 1024], f32r)
    rhat_sb = sb.tile([128, 1024], f32r)
    h_sb = sb.tile([128, 2048], f32r)        # mtile m at cols [512*m]
    f_sb = sb.tile([128, 1024], f32r)         # refined
    sq2_sb = sb.tile([128, 1024], f32r)
    y2_sb = sb.tile([128, 1024], f32r)
    out_sb = sb.tile([128, 1024], f32)       # token tile tt at cols [256*tt]

    var1_sb = sb.tile([1, T], f32)
    rstd1_sb = sb.tile([1, T], f32r)
    q1_sb = sb.tile([1, T], f32r)
    m2_sb = sb.tile([1, T], f32)
    stdc_sb = sb.tile([128, 4], f32)
    rstdc_sb = sb.tile([128, 4], f32)
    var2_sb = sb.tile([1, T], f32)
    rstd2_sb = sb.tile([1, T], f32r)
    q2_sb = sb.tile([1, T], f32r)
    m2b_sb = sb.tile([1, T], f32)
    stdc2_sb = sb.tile([128, 4], f32)
    rstdc2_sb = sb.tile([128, 4], f32)

    dma(xc_sb[:], xc.rearrange("(t p) d -> p t d", p=128))
    dma(xpe_sb[:], xpe.rearrange("(j p) d -> p j d", p=128))
    dma(xpo_sb[:], xpo.rearrange("(j p) d -> p j d", p=128))

    mm = nc.tensor.matmul
    act = nc.scalar.activation
    tt_ = nc.vector.tensor_tensor
    tcp = nc.vector.tensor_copy

    # ---------------- x^T (own) : 8 PE transposes ----------------
    for tt in range(4):
        for dh in range(2):
            p = pt.tile([128, 128], f32, tag="ptr")
            nc.tensor.transpose(p[:], xc_sb[:, 256 * tt + 128 * dh:
                                            256 * tt + 128 * dh + 128],
                                eyef_sb[:])
            tcp(xT_sb[:, 512 * dh + 128 * tt: 512 * dh + 128 * tt + 128], p[:])

    # ---------------- prefix x_even^T : 12 PE transposes ----------------
    for j in range(6):
        for kt in range(2):
            p = pt.tile([128, 128], bf16, tag="ptr")
            nc.tensor.transpose(p[:], xpe_sb[:, 256 * j + 128 * kt:
                                             256 * j + 128 * kt + 128],
                                eyeb_sb[:])
            tcp(xpeT_sb[:, 768 * kt + 128 * j: 768 * kt + 128 * j + 128],
                p[:])

    # ---------------- own phases -> Q ----------------
    ph_ps = pa.tile([P, T], f32, tag="pa")
    for kt in range(2):
        mm(ph_ps[:], _r(kw_sb[:, P * kt: P * kt + P]),
           _r(xT_sb[:, 512 * kt: 512 * kt + 512]),
           start=(kt == 0), stop=(kt == 1))
    act(t_sb[:], ph_ps[:], AF.Tanh, bias=kbc_sb[:])
    act(ta_sb[:], t_sb[:], AF.Abs, bias=zerop_sb[:])
    act(qb_sb[0:P, :], ta_sb[:], AF.Sin, bias=halfpi_sb[:], scale=-PI)
    act(qb_sb[P:2 * P, :], t_sb[:], AF.Sin, bias=zerop_sb[:], scale=PI)

    # ---------------- prefix phases -> Kpre ----------------
    pp1 = pa.tile([P, 512], f32, tag="pa")
    pp2 = pa.tile([P, 256], f32, tag="pa")
    for kt in range(2):
        mm(pp1[:], kwb_sb[:, P * kt: P * kt + P],
           xpeT_sb[:, 768 * kt: 768 * kt + 512],
           start=(kt == 0), stop=(kt == 1))
    for kt in range(2):
        mm(pp2[:], kwb_sb[:, P * kt: P * kt + P],
           xpeT_sb[:, 768 * kt + 512: 768 * kt + 768],
           start=(kt == 0), stop=(kt == 1))
    act(tpre_sb[:, 0:512], pp1[:], AF.Tanh, bias=kbc_sb[:])
    act(tpre_sb[:, 512:768], pp2[:], AF.Tanh, bias=kbc_sb[:])
    act(tpa_sb[:], tpre_sb[:], AF.Abs, bias=zerop_sb[:])
    act(kpre_sb[0:P, :], tpa_sb[:], AF.Sin, bias=halfpi_sb[:], scale=-PI)
    act(kpre_sb[P:2 * P, :], tpre_sb[:], AF.Sin, bias=zerop_sb[:], scale=PI)

    # ---------------- Kpre^T, G, S ----------------
    for j in range(6):
        p = pt.tile([128, 64], bf16, tag="ptr")
        nc.tensor.transpose(p[:], kpre_sb[:, 128 * j: 128 * j + 128],
                            eyeb_sb[0:64, 0:64])
        tcp(kpreT_sb[:, 64 * j: 64 * j + 64], p[:])
    g_ps = pa.tile([2 * P, D], f32, tag="pa")
    for j in range(6):
        mm(g_ps[:], kpreT_sb[:, 64 * j: 64 * j + 64],
           xpo_sb[:, 256 * j: 256 * j + 256],
           start=(j == 0), stop=(j == 5))
    tcp(g_sb[:], g_ps[:])
    for dh in range(2):
        p = pt.tile([128, 64], f32, tag="ptr")
        nc.tensor.transpose(p[:], g_sb[:, 128 * dh: 128 * dh + 128],
                            eyef_sb[0:64, 0:64])
        tcp(gT_sb[:, 64 * dh: 64 * dh + 64], p[:])
    s_ps = pa.tile([2 * P, D], f32, tag="pa")
    for kt in range(2):
        mm(s_ps[:], _r(gT_sb[:, 64 * kt: 64 * kt + 64]),
           _r(vw_sb[:, 256 * kt: 256 * kt + 256]),
           start=(kt == 0), stop=(kt == 1))
    tcp(s_sb[:], s_ps[:])

    # ---------------- V at odd own tokens ----------------
    for blk in range(2):
        vp = pa.tile([128, D], f32, tag="pa")
        for kt in range(2):
            mm(vp[:], _r(xT_sb[:, 512 * kt + 256 * blk + 1:
                               512 * kt + 256 * blk + 256: 2]),
               _r(vw_sb[:, 256 * kt: 256 * kt + 256]),
               start=(kt == 0), stop=(kt == 1))
        tcp(vodd_sb[:, 256 * blk: 256 * blk + 256], vp[:])

    # ---------------- scores (odd tk only) + causal mask ----------------
    sc0 = pb.tile([128, 512], f32, tag="pb")
    mm(sc0[:], _r(qb_sb[:, 0:255:2]), _r(qb_sb[:]), start=True, stop=True)
    tt_(ss0_sb[:, 0:256], sc0[:, 0:256], m0_sb[:], OP.mult)
    tcp(ss0_sb[:, 256:512], sc0[:, 256:512])
    sc1 = pa.tile([128, 256], f32, tag="pa")
    mm(sc1[:], _r(qb_sb[:, 256:511:2]), _r(qb_sb[:, 256:512]),
       start=True, stop=True)
    tt_(ss1_sb[:], sc1[:], m0_sb[:], OP.mult)

    # ---------------- retrieved^T = V^T s + S^T Q ----------------
    retr = []
    for dh in range(2):
        rp = pb.tile([128, 512], f32, tag="pb")
        mm(rp[:], _r(s_sb[:, 128 * dh: 128 * dh + 128]), _r(qb_sb[:]),
           start=True, stop=False)
        mm(rp[:], _r(vodd_sb[:, 128 * dh: 128 * dh + 128]), _r(ss0_sb[:]),
           start=False, stop=False, skip_group_check=True)
        mm(rp[:, 256:512], _r(vodd_sb[:, 256 + 128 * dh: 256 + 128 * dh + 128]),
           _r(ss1_sb[:]), start=False, stop=True, skip_group_check=True)
        retr.append(rp)

    # ---------------- LN1 (folded) ----------------
    def layer_norm(src_ps, src_sb, sqt, stp, msqp, var_sb, m2v_sb, stdc, rstdc,
                   rstd_sb, q_sb, badd):
        # src_ps: 2 psum tiles [128,512] (or None -> read src_sb);
        # copies to src_sb (+ optional per-partition bias), squares, stats,
        # rstd row + broadcast; returns rstdb psum tile [128,512].
        for dh in range(2):
            sl = slice(512 * dh, 512 * dh + 512)
            if src_ps is not None:
                if badd is None:
                    tcp(src_sb[:, sl], src_ps[dh][:])
                    act(sqt[:, sl], src_ps[dh][:], AF.Square, bias=zero128_sb[:])
                else:
                    nc.vector.tensor_scalar(
                        out=src_sb[:, sl], in0=src_ps[dh][:],
                        scalar1=badd[:, dh: dh + 1], scalar2=None, op0=OP.add)
                    act(sqt[:, sl], src_sb[:, sl], AF.Square, bias=zero128_sb[:])
            else:
                act(sqt[:, sl], src_sb[:, sl], AF.Square, bias=zero128_sb[:])
        for kt in range(2):
            mm(stp[0:1, :], _r(invd_sb[:, 0:1]),
               _r(src_sb[:, 512 * kt: 512 * kt + 512]),
               start=(kt == 0), stop=(kt == 1))
        for kt in range(2):
            mm(msqp[0:1, :], _r(invd_sb[:, 0:1]),
               _r(sqt[:, 512 * kt: 512 * kt + 512]),
               start=(kt == 0), stop=(kt == 1))
        act(m2v_sb[:], stp[0:1, :], AF.Square, bias=zero1_sb[:])
        tt_(var_sb[:], msqp[0:1, :], m2v_sb[:], OP.subtract)
        vc = pt.tile([128, 4], f32, tag="ptr")
        for j in range(4):
            nc.tensor.transpose(vc[:, j: j + 1],
                                var_sb[0:1, 128 * j: 128 * j + 128],
                                eyef_sb[0:1, 0:1])
        act(stdc[:], vc[:], AF.Sqrt, bias=epsb_sb[:])
        nc.vector.reciprocal(rstdc[:], stdc[:])
        rr = pa.tile([1, T], f32, tag="pa")
        for j in range(4):
            nc.tensor.transpose(rr[0:1, 128 * j: 128 * j + 128],
                                rstdc[:, j: j + 1], eyef_sb[:])
        tcp(rstd_sb[:], rr[:])
        tt_(q_sb[:], stp[0:1, :], rstd_sb[:], OP.mult)
        rb = pb.tile([128, 512], f32, tag="pb")
        mm(rb[:], _r(ones_sb[:]), _r(rstd_sb[:]), start=True, stop=True)
        return rb

    st1 = pa.tile([1, T], f32, tag="pa")
    ms1 = pa.tile([1, T], f32, tag="pa2", bufs=1)
    rb1 = layer_norm(retr, r_sb, sq_sb, st1, ms1, var1_sb, m2_sb, stdc_sb,
                     rstdc_sb, rstd1_sb, q1_sb, None)
    for dh in range(2):
        sl = slice(512 * dh, 512 * dh + 512)
        tt_(rhat_sb[:, sl], r_sb[:, sl], rb1[:], OP.mult)

    # ---------------- W1 + rank-1 mean fix + GELU ----------------
    for m in range(4):
        hp = pb.tile([128, 512], f32, tag="pb")
        for kt in range(2):
            mm(hp[:], w1g_sb[:, 512 * kt + 128 * m: 512 * kt + 128 * m + 128],
               rhat_sb[:, 512 * kt: 512 * kt + 512],
               start=(kt == 0), stop=False)
        mm(hp[:], c1n_sb[0:1, 128 * m: 128 * m + 128], q1_sb[:],
           start=False, stop=True, skip_group_check=True)
        act(h_sb[:, 512 * m: 512 * m + 512], hp[:], AF.Gelu,
            bias=cb1_sb[:, m: m + 1])

    # ---------------- W2 -> refined ----------------
    ref = []
    for dh in range(2):
        fp = pb.tile([128, 512], f32, tag="pb")
        for kt in range(4):
            mm(fp[:], w2_sb[:, 256 * kt + 128 * dh: 256 * kt + 128 * dh + 128],
               h_sb[:, 512 * kt: 512 * kt + 512],
               start=(kt == 0), stop=(kt == 3))
        ref.append(fp)

    # ---------------- LN2 (folded) ----------------
    st2 = pa.tile([1, T], f32, tag="pa")
    ms2 = pa.tile([1, T], f32, tag="pa2", bufs=1)
    rb2 = layer_norm(ref, f_sb, sq2_sb, st2, ms2, var2_sb, m2b_sb, stdc2_sb,
                     rstdc2_sb, rstd2_sb, q2_sb, b2c_sb)
    for dh in range(2):
        sl = slice(512 * dh, 512 * dh + 512)
        tt_(y2_sb[:, sl], f_sb[:, sl], rb2[:], OP.mult)

    # ---------------- Wo (token-major) + rank-1 + residual ----------------
    for tm in range(4):
        op = pa.tile([128, D], f32, tag="pa")
        for dh in range(2):
            mm(op[:], y2_sb[:, 512 * dh + 128 * tm: 512 * dh + 128 * tm + 128],
               wog_sb[:, 256 * dh: 256 * dh + 256],
               start=(dh == 0), stop=False)
        mm(op[:], q2_sb[0:1, 128 * tm: 128 * tm + 128], c1on_sb[:],
           start=False, stop=True, skip_group_check=True)
        tt_(out_sb[:, 256 * tm: 256 * tm + 256], op[:],
            xc_sb[:, 256 * tm: 256 * tm + 256], OP.add)

    dma(outc.rearrange("(t p) d -> p t d", p=128), out_sb[:])

    pb.release()
    pa.release()
    pt.release()
    sb.release()


_CACHE = {}


def _get_nc():
    if "nc" not in _CACHE:
        _CACHE["nc"] = _build()
    return _CACHE["nc"]


def _bf(a):
    return np.asarray(a, np.float32).astype(ml_dtypes.bfloat16)


def kernel(**inputs):
    x = np.asarray(inputs["x"], np.float32)
    key_W = np.asarray(inputs["key_W"], np.float32)
    key_b = np.asarray(inputs["key_b"], np.float32)
    val_W = np.asarray(inputs["val_W"], np.float32)
    val_b = np.asarray(inputs["val_b"], np.float32)
    ln1_g = np.asarray(inputs["ln1_g"], np.float32)
    ln1_b = np.asarray(inputs["ln1_b"], np.float32)
    W1 = np.asarray(inputs["W1"], np.float32)
    b1 = np.asarray(inputs["b1"], np.float32)
    W2 = np.asarray(inputs["W2"], np.float32)
    b2 = np.asarray(inputs["b2"], np.float32)
    ln2_g = np.asarray(inputs["ln2_g"], np.float32)
    ln2_b = np.asarray(inputs["ln2_b"], np.float32)
    Wo = np.asarray(inputs["Wo"], np.float32)
    bo = np.asarray(inputs["bo"], np.float32)

    # these are identically zero for this module; the kernel folds them out
    assert np.allclose(val_b, 0.0), "nonzero val_b unsupported"
    assert np.allclose(bo + ln2_b @ Wo, 0.0), "nonzero output bias unsupported"

    w1g = ln1_g[:, None] * W1
    wog = ln2_g[:, None] * Wo
    shared = {
        "kw": key_W, "kwb": _bf(key_W), "kbc": key_b.reshape(P, 1),
        "vw": val_W,
        "w1g": w1g, "c1n": -w1g.sum(0, keepdims=True),
        "cb1": (b1 + ln1_b @ W1).reshape(H, 1),
        "w2": W2, "b2c": b2.reshape(D, 1),
        "wog": wog, "c1on": -wog.sum(0, keepdims=True),
        "m0": (np.arange(1, 256, 2)[:, None] <=
               np.arange(256)[None, :]).astype(np.float32),
        "eyef": np.eye(128, dtype=np.float32),
        "onesr": np.ones((1, 128), np.float32),
        "invd": np.full((D, 1), 1.0 / D, np.float32)[:128],
        "eyeb": _bf(np.eye(128)),
    }
    in_maps = []
    for i in range(N_CORES):
        b, c = divmod(i, 4)
        l0 = c * T
        npairs = l0 // 2
        xpe = np.zeros((PRE, D), np.float32)
        xpo = np.zeros((PRE, D), np.float32)
        if npairs:
            xpe[:npairs] = x[b, 0:l0 - 1:2]
            xpo[:npairs] = x[b, 1:l0:2]
        in_maps.append({
            "xc": np.ascontiguousarray(x[b, l0:l0 + T]),
            "xpe": _bf(xpe), "xpo": _bf(xpo), **shared,
        })

    nc = _get_nc()
    res = run_bass_kernel_spmd(nc, in_maps, core_ids=list(range(N_CORES)),
                               **_CACHE.get("run_kwargs", {}))
    _CACHE["last_result"] = res
    out = np.empty((B, L, D), np.float32)
    for i in range(N_CORES):
        b, c = divmod(i, 4)
        out[b, c * T:(c + 1) * T] = res.results[i]["outc"]
    return out
